# revision 4
# baseline (speedup 1.0000x reference)
"""Trainium2 Bass kernel v4 for nn_MicrobiomeTreeModel.

Tree MLP over B=524288 samples == 4 chained matmuls with block-diagonal
weights A1 [16,128], A2 [128,64], A3 [64,32], A4 [32,16], ReLU between.

v4 vs v3 (sim 141us, ACT 81%/DVE 64%/PE 64%):
  - input transpose moved to the DMA xbar (4 transpose-DMAs per super on
    HWDGE; xin is bf16 via the gpsimd cast-DMA load) — frees the PE
    in-transposes, the psxt bank, and the DVE xT copy
  - psmid pairs: two rounds' L2/L3L4 matmuls land in one [128,1024] 2-bank
    PSUM tile, drained by ONE ACT op (FD amortizes: 570 -> 498 ns/unit)
  - psout pairs: two units' out tiles drained by one ACT op (197 -> 125)
  - outstage is fp32: the bf16->fp32 cast rides the ACT out-drain for
    free, so the final out DMA is plain HWDGE (no SWDGE descgen cost)

Steady-state per unit: PE 3 matmuls + 4 tiny transposes (~670ns),
DVE h1 drain (658ns), ACT mid+out drains (~620ns).

Unit u schedule (round r):
  r=u      L1 -> psh1[u]
  r=u+1    DVE drain psh1 -> h1sb[u] (bias+relu, bf16)
  r=u+3    L2 -> psmid-pair[.][0:64]; L3+L4 fused -> [64:112]
  r=pair+2 ACT drain psmid pair -> midsb pair (bias+relu)
  r=u+13   (u even) 8 PE transposes midsb[96:112] units u,u+1 -> psout,
           ACT copy pair -> outstage (fp32)
"""

import numpy as np

B = 524288
NCORES = 8
BC = B // NCORES  # 65536 samples per core
H = 16

UNIT = 512
UNITS = BC // UNIT  # 128
SUPER = 8  # units per xT staging tile (4096 samples)
GROUP = 32  # units per DMA group (16384 samples = 1 MiB fp32)

_CACHE = {}


def _build_nc(units=UNITS, repeats=1):
    import concourse.tile as tile
    from concourse import bacc, mybir

    bc = units * UNIT
    group = min(GROUP, units)
    nsup = max(group // SUPER, 1)  # supers per group
    nsupers = max(units // SUPER, 1)
    ngroups = max(units // group, 1)
    jg = group * 4  # samples per chunk-partition per group

    f32 = mybir.dt.float32
    bf16 = mybir.dt.bfloat16
    RELU = mybir.ActivationFunctionType.Relu
    COPY = mybir.ActivationFunctionType.Copy
    ADD = mybir.AluOpType.add
    MAX = mybir.AluOpType.max

    nc = bacc.Bacc("TRN2", target_bir_lowering=False)

    x_in = nc.dram_tensor("x", [bc, 16], f32, kind="ExternalInput")
    a1lo_in = nc.dram_tensor("a1lo", [128, 128], bf16, kind="ExternalInput")
    a1hi_in = nc.dram_tensor("a1hi", [128, 128], bf16, kind="ExternalInput")
    a2_in = nc.dram_tensor("a2", [128, 64], bf16, kind="ExternalInput")
    a34_in = nc.dram_tensor("a34", [96, 48], bf16, kind="ExternalInput")
    i16_in = nc.dram_tensor("i16", [16, 16], bf16, kind="ExternalInput")
    i128_in = nc.dram_tensor("i128", [128, 128], bf16, kind="ExternalInput")
    b1_in = nc.dram_tensor("b1v", [128, 1], f32, kind="ExternalInput")
    b2_in = nc.dram_tensor("b2v", [112, 1], f32, kind="ExternalInput")
    out_dram = nc.dram_tensor("out", [bc, 16], f32, kind="ExternalOutput")

    # partition p holds jg consecutive samples: contiguous 64B*jg runs
    x_view = x_in[:].rearrange("(G p j) f -> G p (j f)", p=128, j=jg)
    out_view = out_dram[:].rearrange("(G p j) f -> G p (j f)", p=128, j=jg)

    with tile.TileContext(nc) as tc:
        with (
            tc.tile_pool(name="wpool", bufs=1) as wpool,
            tc.tile_pool(name="xinp", bufs=2) as xinp,
            tc.tile_pool(name="xtp", bufs=3) as xtp,
            tc.tile_pool(name="opool", bufs=2) as opool,
            tc.tile_pool(name="h1p", bufs=4) as h1p,
            tc.tile_pool(name="midp", bufs=4) as midp,
            tc.tile_pool(name="ps_h1", bufs=3, space="PSUM") as ps_h1,
            tc.tile_pool(name="ps_mid", bufs=2, space="PSUM") as ps_mid,
            tc.tile_pool(name="ps_o", bufs=1, space="PSUM") as ps_o,
        ):
            # ---- startup: a1lo + identity first, then the group-0
            # bootstrap loads (HWDGE fp32, bypassing SWDGE descgen),
            # then the remaining weights ----
            a1lo = wpool.tile([128, 128], bf16, tag="a1lo")
            nc.sync.dma_start(a1lo[:], a1lo_in[:])
            i128 = wpool.tile([128, 128], bf16, tag="i128")
            nc.sync.dma_start(i128[:], i128_in[:])
            xboot0 = wpool.tile([128, jg * 8], f32, tag="xboot0")
            nc.sync.dma_start(xboot0[:], x_view[0][:, 0 : jg * 8])
            xboot1 = wpool.tile([128, jg * 8], f32, tag="xboot1")
            nc.sync.dma_start(xboot1[:], x_view[0][:, jg * 8 : jg * 16])
            a1hi = wpool.tile([128, 128], bf16, tag="a1hi")
            nc.sync.dma_start(a1hi[:], a1hi_in[:])
            a2w = wpool.tile([128, 64], bf16, tag="a2")
            nc.sync.dma_start(a2w[:], a2_in[:])
            a34w = wpool.tile([96, 48], bf16, tag="a34")
            nc.sync.dma_start(a34w[:], a34_in[:])
            i16b = wpool.tile([112, 16], bf16, tag="i16")
            nc.sync.dma_start(i16b[96:112, :], i16_in[:])
            b1v = wpool.tile([128, 1], f32, tag="b1v")
            nc.sync.dma_start(b1v[:], b1_in[:])
            b2v = wpool.tile([112, 1], f32, tag="b2v")
            nc.sync.dma_start(b2v[:], b2_in[:])

            def load_group(gi):
                t = xinp.tile([128, jg * 16], bf16, tag="xin")
                if gi == 0:
                    nc.vector.tensor_copy(t[:, 0 : jg * 8], xboot0[:])
                    nc.vector.tensor_copy(t[:, jg * 8 : jg * 16], xboot1[:])
                else:
                    nc.gpsimd.dma_start(t[:], x_view[gi])
                return t

            def stage_super(st, xin_t):
                # xT row 16*j''+f, col (b,c): x[sample c*jg + jwin+8b+j'', f]
                base = 512 * (st % nsup)
                t = xtp.tile([128, 512], bf16, tag="xT")
                if st < 2:
                    # bootstrap: PE transposes (HWDGE is busy with weights;
                    # borrow the ps_o bank, idle until the first out quad)
                    psxt = ps_o.tile([128, 512], bf16, tag="o", name="psxtb")
                    for b in range(4):
                        nc.tensor.transpose(
                            psxt[:, 128 * b : 128 * (b + 1)],
                            xin_t[:, base + 128 * b : base + 128 * (b + 1)],
                            i128[:],
                        )
                    nc.vector.tensor_copy(t[:], psxt[:])
                    return t
                for b in range(4):
                    nc.sync.dma_start(
                        t[:, 128 * b : 128 * (b + 1)],
                        xin_t[:, base + 128 * b : base + 128 * (b + 1)],
                        transpose=True,
                    )
                return t

            xin_cur = load_group(0)
            xin_next = xin_cur
            xt_next = stage_super(0, xin_cur)
            xt_cur = None
            outn_t = None

            h1hist = {}
            midhist = {}
            psh1_hist = {}
            psmid_pair = {}  # q -> (tile, {half: (lo, hi)})

            def mid_rows(r):
                l2 = 0 <= r - 4 < units
                l3 = 0 <= r - 9 < units
                l4 = 0 <= r - 14 < units
                if not (l2 or l3 or l4):
                    return None
                lo = 0 if l2 else (64 if l3 else 96)
                hi = 112 if l4 else (96 if l3 else 64)
                return lo, hi

            for _rep in range(repeats):
                for r in range(units + 23):
                    u = r
                    if u < units:
                        if u % group == 0:
                            gi1 = u // group + 1
                            if gi1 < ngroups:
                                xin_next = load_group(gi1)
                        if u % SUPER == 0:
                            xt_cur = xt_next
                            st = u // SUPER + 1
                            if st < nsupers:
                                if st % nsup == 0:
                                    xin_cur = xin_next
                                xt_next = stage_super(st, xin_cur)
                        # ---- L1 ----
                        k = u % SUPER
                        g, parity = k // 2, k % 2
                        a1w = a1hi if parity else a1lo
                        psh1 = ps_h1.tile([128, 512], f32, tag="h1")
                        nc.tensor.matmul(
                            psh1[:],
                            a1w[32 * g : 32 * (g + 1), :],
                            xt_cur[32 * g : 32 * (g + 1), :],
                            start=True,
                            stop=True,
                            tile_position=(32 * g, 0),
                        )
                        psh1_hist[u] = psh1

                    # ---- h1 drain (lag 1) ----
                    ud = r - 1
                    if 0 <= ud < units:
                        h1sb = h1p.tile([128, 512], bf16, tag="h1s")
                        nc.vector.tensor_scalar(
                            h1sb[:], psh1_hist.pop(ud)[:], b1v[:, 0:1], 0.0,
                            ADD, MAX,
                        )
                        h1hist[ud] = h1sb

                    # ---- L2 (lag 3) + fused L3/L4 into psmid pair ----
                    rows = mid_rows(r)
                    if rows is not None:
                        q, half = r // 2, r % 2
                        if q not in psmid_pair:
                            psmid_pair[q] = (
                                ps_mid.tile([128, 1024], f32, tag="mid",
                                            name="psmidp"),
                                {},
                            )
                        pt, halves = psmid_pair[q]
                        halves[half] = rows
                        col = 512 * half
                        c2, c3, c4 = r - 4, r - 9, r - 14
                        if 0 <= c2 < units:
                            nc.tensor.matmul(
                                pt[0:64, col : col + 512], a2w[:],
                                h1hist.pop(c2)[:],
                                start=True, stop=True,
                            )
                        l3 = 0 <= c3 < units
                        l4 = 0 <= c4 < units
                        if l3 or l4:
                            mt, mh = midhist[r - 5]
                            mc = 512 * mh
                            if l3 and l4:
                                nc.tensor.matmul(
                                    pt[64:112, col : col + 512], a34w[:],
                                    mt[0:96, mc : mc + 512],
                                    start=True, stop=True,
                                    tile_position=(0, 64),
                                )
                            elif l3:
                                nc.tensor.matmul(
                                    pt[64:96, col : col + 512],
                                    a34w[0:64, 0:32],
                                    mt[0:64, mc : mc + 512],
                                    start=True, stop=True,
                                    tile_position=(0, 64),
                                )
                            else:
                                nc.tensor.matmul(
                                    pt[96:112, col : col + 512],
                                    a34w[64:96, 32:48],
                                    mt[64:96, mc : mc + 512],
                                    start=True, stop=True,
                                    tile_position=(64, 96),
                                )

                    # ---- mid pair drain (at round 2q+2 for pair q) ----
                    qd = r // 2 - 1
                    if r % 2 == 0 and qd in psmid_pair:
                        pt, halves = psmid_pair.pop(qd)
                        midsb = midp.tile([112, 1024], bf16, tag="mids")
                        items = sorted(halves.items())
                        if len(items) == 2 and items[0][1] == items[1][1]:
                            lo, hi = items[0][1]
                            if r > units + 2 and qd % 2 == 1:
                                # epilogue: DVE is idle once h1 drains end;
                                # alternating engines overlaps pair drains
                                nc.vector.tensor_scalar(
                                    midsb[lo:hi, :], pt[lo:hi, :],
                                    b2v[lo:hi, 0:1], 0.0, ADD, MAX,
                                )
                            else:
                                nc.scalar.activation(
                                    midsb[lo:hi, :], pt[lo:hi, :],
                                    RELU, bias=b2v[lo:hi, 0:1],
                                )
                        else:
                            for half, (lo, hi) in items:
                                col = 512 * half
                                nc.scalar.activation(
                                    midsb[lo:hi, col : col + 512],
                                    pt[lo:hi, col : col + 512],
                                    RELU, bias=b2v[lo:hi, 0:1],
                                )
                        midhist[2 * qd] = (midsb, 0)
                        midhist[2 * qd + 1] = (midsb, 1)
                        for old in (2 * qd - 8, 2 * qd - 7):
                            midhist.pop(old, None)

                    # ---- out batch: whole super (v%8==0) at round v+23;
                    # 32 transposes per burst amortize the PE
                    # matmul<->transpose mode-switch cost (~160ns/op in
                    # 4-op bursts, ~3ns in large clean runs) ----
                    v = r - 23
                    if 0 <= v < units and v % SUPER == 0:
                        if v % group == 0:
                            outn_t = opool.tile([128, jg * 16], f32, tag="outn")
                        psout = ps_o.tile([128, 512], bf16, tag="o")
                        for up in range(8):
                            mt, mh = midhist[v + up + 14]
                            mc = 512 * mh
                            for b in range(4):
                                nc.tensor.transpose(
                                    psout[
                                        :,
                                        128 * b + 16 * up : 128 * b + 16 * up + 16,
                                    ],
                                    mt[96:112, mc + 128 * b : mc + 128 * (b + 1)],
                                    i16b[96:112, :],
                                    tile_position=(96, 0),
                                )
                        # outn col = j*16+f, j = S*32 + b*8 + j2
                        s_v = v // SUPER
                        dst = outn_t[:].rearrange(
                            "p (S b j2 f) -> p S b j2 f", b=4, j2=8, f=16
                        )[:, s_v % nsup, :, :, :]
                        src = psout[:].rearrange("p (b u f) -> p b u f", b=4, u=8)
                        if (v // 8) % 2 == 0:
                            nc.scalar.activation(dst, src, COPY)
                        else:
                            nc.vector.tensor_copy(dst, src)
                        go = v // group
                        if go == ngroups - 1 and ngroups > 1 and group > SUPER:
                            nc.sync.dma_start(
                                out_view[go][
                                    :,
                                    512 * (s_v % nsup) : 512 * (s_v % nsup) + 512,
                                ],
                                outn_t[:, 512 * (s_v % nsup) : 512 * (s_v % nsup) + 512],
                            )
                        elif (v + 7) % group == group - 1:
                            nc.sync.dma_start(out_view[go], outn_t[:])

    nc.compile()
    return nc


def _prep_weights(W1, b1, W2, b2, W3, b3, W4, b4):
    import ml_dtypes

    bf16 = ml_dtypes.bfloat16
    A1 = np.zeros((16, 128), np.float32)
    for n in range(8):
        A1[2 * n : 2 * n + 2, 16 * n : 16 * n + 16] = W1[n]
    a1lo = np.zeros((128, 128), np.float32)
    a1hi = np.zeros((128, 128), np.float32)
    for g in range(4):
        a1lo[32 * g : 32 * g + 16, :] = A1
        a1hi[32 * g + 16 : 32 * g + 32, :] = A1
    A2 = np.zeros((128, 64), np.float32)
    for n in range(4):
        A2[32 * n : 32 * n + 32, 16 * n : 16 * n + 16] = W2[n]
    A34 = np.zeros((96, 48), np.float32)
    for n in range(2):
        A34[32 * n : 32 * n + 32, 16 * n : 16 * n + 16] = W3[n]
    A34[64:96, 32:48] = W4
    b1v = np.ascontiguousarray(b1.reshape(128, 1), dtype=np.float32)
    b2v = np.concatenate(
        [b2.reshape(-1), b3.reshape(-1), b4.reshape(-1)]
    ).reshape(112, 1).astype(np.float32)
    return {
        "a1lo": a1lo.astype(bf16),
        "a1hi": a1hi.astype(bf16),
        "a2": A2.astype(bf16),
        "a34": A34.astype(bf16),
        "i16": np.eye(16, dtype=np.float32).astype(bf16),
        "i128": np.eye(128, dtype=np.float32).astype(bf16),
        "b1v": b1v,
        "b2v": b2v,
    }


def kernel(x, W1, b1, W2, b2, W3, b3, W4, b4, **_unused):
    from concourse.bass_utils import run_bass_kernel_spmd

    if "nc" not in _CACHE:
        _CACHE["nc"] = _build_nc()
    nc = _CACHE["nc"]

    x = np.ascontiguousarray(np.asarray(x, dtype=np.float32))
    wmap = _prep_weights(
        np.asarray(W1, np.float32),
        np.asarray(b1, np.float32),
        np.asarray(W2, np.float32),
        np.asarray(b2, np.float32),
        np.asarray(W3, np.float32),
        np.asarray(b3, np.float32),
        np.asarray(W4, np.float32),
        np.asarray(b4, np.float32),
    )
    in_maps = [
        {"x": x[i * BC : (i + 1) * BC], **wmap} for i in range(NCORES)
    ]
    _CACHE["in_maps"] = in_maps
    res = run_bass_kernel_spmd(nc, in_maps, core_ids=list(range(NCORES)))
    _CACHE["last"] = res
    out = np.concatenate([res.results[i]["out"] for i in range(NCORES)], axis=0)
    return out.astype(np.float32)



# revision 7
# speedup vs baseline: 1.1047x; 1.1047x over previous
"""Trainium2 Bass kernel v4 for nn_MicrobiomeTreeModel.

Tree MLP over B=524288 samples == 4 chained matmuls with block-diagonal
weights A1 [16,128], A2 [128,64], A3 [64,32], A4 [32,16], ReLU between.

v4 vs v3 (sim 141us, ACT 81%/DVE 64%/PE 64%):
  - input transpose moved to the DMA xbar (4 transpose-DMAs per super on
    HWDGE; xin is bf16 via the gpsimd cast-DMA load) — frees the PE
    in-transposes, the psxt bank, and the DVE xT copy
  - psmid pairs: two rounds' L2/L3L4 matmuls land in one [128,1024] 2-bank
    PSUM tile, drained by ONE ACT op (FD amortizes: 570 -> 498 ns/unit)
  - psout pairs: two units' out tiles drained by one ACT op (197 -> 125)
  - outstage is fp32: the bf16->fp32 cast rides the ACT out-drain for
    free, so the final out DMA is plain HWDGE (no SWDGE descgen cost)

Steady-state per unit: PE 3 matmuls + 4 tiny transposes (~670ns),
DVE h1 drain (658ns), ACT mid+out drains (~620ns).

Unit u schedule (round r):
  r=u      L1 -> psh1[u]
  r=u+1    DVE drain psh1 -> h1sb[u] (bias+relu, bf16)
  r=u+3    L2 -> psmid-pair[.][0:64]; L3+L4 fused -> [64:112]
  r=pair+2 ACT drain psmid pair -> midsb pair (bias+relu)
  r=u+13   (u even) 8 PE transposes midsb[96:112] units u,u+1 -> psout,
           ACT copy pair -> outstage (fp32)
"""

import numpy as np

B = 524288
NCORES = 8
BC = B // NCORES  # 65536 samples per core
H = 16

UNIT = 512
UNITS = BC // UNIT  # 128
SUPER = 8  # units per xT staging tile (4096 samples)
GROUP = 32  # units per DMA group (16384 samples = 1 MiB fp32)

_CACHE = {}


def _build_nc(units=UNITS, repeats=1):
    import concourse.tile as tile
    from concourse import bacc, mybir

    bc = units * UNIT
    group = min(GROUP, units)
    nsup = max(group // SUPER, 1)  # supers per group
    nsupers = max(units // SUPER, 1)
    ngroups = max(units // group, 1)
    jg = group * 4  # samples per chunk-partition per group

    f32 = mybir.dt.float32
    bf16 = mybir.dt.bfloat16
    RELU = mybir.ActivationFunctionType.Relu
    COPY = mybir.ActivationFunctionType.Copy
    ADD = mybir.AluOpType.add
    MAX = mybir.AluOpType.max

    nc = bacc.Bacc("TRN2", target_bir_lowering=False)

    x_in = nc.dram_tensor("x", [bc, 16], f32, kind="ExternalInput")
    a1lo_in = nc.dram_tensor("a1lo", [128, 128], bf16, kind="ExternalInput")
    a1hi_in = nc.dram_tensor("a1hi", [128, 128], bf16, kind="ExternalInput")
    a2_in = nc.dram_tensor("a2", [128, 64], bf16, kind="ExternalInput")
    a34_in = nc.dram_tensor("a34", [96, 48], bf16, kind="ExternalInput")
    i16_in = nc.dram_tensor("i16", [16, 16], bf16, kind="ExternalInput")
    i128_in = nc.dram_tensor("i128", [128, 128], bf16, kind="ExternalInput")
    b1_in = nc.dram_tensor("b1v", [128, 1], f32, kind="ExternalInput")
    b2_in = nc.dram_tensor("b2v", [112, 1], f32, kind="ExternalInput")
    out_dram = nc.dram_tensor("out", [bc, 16], f32, kind="ExternalOutput")

    # partition p holds jg consecutive samples: contiguous 64B*jg runs
    x_view = x_in[:].rearrange("(G p j) f -> G p (j f)", p=128, j=jg)
    out_view = out_dram[:].rearrange("(G p j) f -> G p (j f)", p=128, j=jg)

    with tile.TileContext(nc) as tc:
        with (
            tc.tile_pool(name="wpool", bufs=1) as wpool,
            tc.tile_pool(name="xinp", bufs=2) as xinp,
            tc.tile_pool(name="xtp", bufs=3) as xtp,
            tc.tile_pool(name="opool", bufs=2) as opool,
            tc.tile_pool(name="h1p", bufs=4) as h1p,
            tc.tile_pool(name="midp", bufs=4) as midp,
            tc.tile_pool(name="ps_h1", bufs=3, space="PSUM") as ps_h1,
            tc.tile_pool(name="ps_mid", bufs=2, space="PSUM") as ps_mid,
            tc.tile_pool(name="ps_o", bufs=1, space="PSUM") as ps_o,
        ):
            # ---- startup: a1lo + identity first, then the group-0
            # bootstrap loads (HWDGE fp32, bypassing SWDGE descgen),
            # then the remaining weights ----
            a1lo = wpool.tile([128, 128], bf16, tag="a1lo")
            nc.sync.dma_start(a1lo[:], a1lo_in[:])
            i128 = wpool.tile([128, 128], bf16, tag="i128")
            nc.sync.dma_start(i128[:], i128_in[:])
            a1hi = wpool.tile([128, 128], bf16, tag="a1hi")
            nc.sync.dma_start(a1hi[:], a1hi_in[:])
            a2w = wpool.tile([128, 64], bf16, tag="a2")
            nc.sync.dma_start(a2w[:], a2_in[:])
            a34w = wpool.tile([96, 48], bf16, tag="a34")
            nc.sync.dma_start(a34w[:], a34_in[:])
            i16b = wpool.tile([112, 16], bf16, tag="i16")
            nc.sync.dma_start(i16b[96:112, :], i16_in[:])
            b1v = wpool.tile([128, 1], f32, tag="b1v")
            nc.sync.dma_start(b1v[:], b1_in[:])
            b2v = wpool.tile([112, 1], f32, tag="b2v")
            nc.sync.dma_start(b2v[:], b2_in[:])

            def load_group(gi):
                t = xinp.tile([128, jg * 16], bf16, tag="xin")
                if gi == 0:
                    # boot: cast sub-loads so super 0 is ready early without
                    # burning DVE on xboot copies (the copies cost 4.3us of
                    # DVE time PER REP in the repeated NEFF)
                    step = (jg * 16) // 4
                    for s in range(4):
                        nc.gpsimd.dma_start(
                            t[:, step * s : step * (s + 1)],
                            x_view[0][:, step * s : step * (s + 1)],
                        )
                else:
                    nc.gpsimd.dma_start(t[:], x_view[gi])
                return t

            def stage_super(st, xin_t):
                # xT row 16*j''+f, col (b,c): x[sample c*jg + jwin+8b+j'', f]
                base = 512 * (st % nsup)
                t = xtp.tile([128, 512], bf16, tag="xT")
                for b in range(4):
                    nc.sync.dma_start(
                        t[:, 128 * b : 128 * (b + 1)],
                        xin_t[:, base + 128 * b : base + 128 * (b + 1)],
                        transpose=True,
                    )
                return t

            xin_cur = load_group(0)
            xin_next = xin_cur
            xt_next = stage_super(0, xin_cur)
            xt_cur = None
            outn_t = None

            h1hist = {}
            midhist = {}
            psh1_hist = {}
            psmid_pair = {}  # q -> (tile, {half: (lo, hi)})

            def mid_rows(r):
                l2 = 0 <= r - 4 < units
                l3 = 0 <= r - 9 < units
                l4 = 0 <= r - 14 < units
                if not (l2 or l3 or l4):
                    return None
                lo = 0 if l2 else (64 if l3 else 96)
                hi = 112 if l4 else (96 if l3 else 64)
                return lo, hi

            for _rep in range(repeats):
                for r in range(units + 23):
                    u = r
                    if u < units:
                        if u % group == 0:
                            gi1 = u // group + 1
                            if gi1 < ngroups:
                                xin_next = load_group(gi1)
                        if u % SUPER == 0:
                            xt_cur = xt_next
                            st = u // SUPER + 1
                            if st < nsupers:
                                if st % nsup == 0:
                                    xin_cur = xin_next
                                xt_next = stage_super(st, xin_cur)
                        # ---- L1 ----
                        k = u % SUPER
                        g, parity = k // 2, k % 2
                        a1w = a1hi if parity else a1lo
                        psh1 = ps_h1.tile([128, 512], f32, tag="h1")
                        nc.tensor.matmul(
                            psh1[:],
                            a1w[32 * g : 32 * (g + 1), :],
                            xt_cur[32 * g : 32 * (g + 1), :],
                            start=True,
                            stop=True,
                            tile_position=(32 * g, 0),
                        )
                        psh1_hist[u] = psh1

                    # ---- h1 drain (lag 1) ----
                    ud = r - 1
                    if 0 <= ud < units:
                        h1sb = h1p.tile([128, 512], bf16, tag="h1s")
                        nc.vector.tensor_scalar(
                            h1sb[:], psh1_hist.pop(ud)[:], b1v[:, 0:1], 0.0,
                            ADD, MAX,
                        )
                        h1hist[ud] = h1sb

                    # ---- L2 (lag 3) + fused L3/L4 into psmid pair ----
                    rows = mid_rows(r)
                    if rows is not None:
                        q, half = r // 2, r % 2
                        if q not in psmid_pair:
                            psmid_pair[q] = (
                                ps_mid.tile([128, 1024], f32, tag="mid",
                                            name="psmidp"),
                                {},
                            )
                        pt, halves = psmid_pair[q]
                        halves[half] = rows
                        col = 512 * half
                        c2, c3, c4 = r - 4, r - 9, r - 14
                        if 0 <= c2 < units:
                            nc.tensor.matmul(
                                pt[0:64, col : col + 512], a2w[:],
                                h1hist.pop(c2)[:],
                                start=True, stop=True,
                            )
                        l3 = 0 <= c3 < units
                        l4 = 0 <= c4 < units
                        if l3 or l4:
                            mt, mh = midhist[r - 5]
                            mc = 512 * mh
                            if l3 and l4:
                                nc.tensor.matmul(
                                    pt[64:112, col : col + 512], a34w[:],
                                    mt[0:96, mc : mc + 512],
                                    start=True, stop=True,
                                    tile_position=(0, 64),
                                )
                            elif l3:
                                nc.tensor.matmul(
                                    pt[64:96, col : col + 512],
                                    a34w[0:64, 0:32],
                                    mt[0:64, mc : mc + 512],
                                    start=True, stop=True,
                                    tile_position=(0, 64),
                                )
                            else:
                                nc.tensor.matmul(
                                    pt[96:112, col : col + 512],
                                    a34w[64:96, 32:48],
                                    mt[64:96, mc : mc + 512],
                                    start=True, stop=True,
                                    tile_position=(64, 96),
                                )

                    # ---- mid pair drain (at round 2q+2 for pair q) ----
                    qd = r // 2 - 1
                    if r % 2 == 0 and qd in psmid_pair:
                        pt, halves = psmid_pair.pop(qd)
                        midsb = midp.tile([112, 1024], bf16, tag="mids")
                        items = sorted(halves.items())
                        if len(items) == 2 and items[0][1] == items[1][1]:
                            lo, hi = items[0][1]
                            if r > units + 2 and qd % 2 == 1:
                                # epilogue: DVE is idle once h1 drains end;
                                # alternating engines overlaps pair drains
                                nc.vector.tensor_scalar(
                                    midsb[lo:hi, :], pt[lo:hi, :],
                                    b2v[lo:hi, 0:1], 0.0, ADD, MAX,
                                )
                            else:
                                nc.scalar.activation(
                                    midsb[lo:hi, :], pt[lo:hi, :],
                                    RELU, bias=b2v[lo:hi, 0:1],
                                )
                        else:
                            for half, (lo, hi) in items:
                                col = 512 * half
                                nc.scalar.activation(
                                    midsb[lo:hi, col : col + 512],
                                    pt[lo:hi, col : col + 512],
                                    RELU, bias=b2v[lo:hi, 0:1],
                                )
                        midhist[2 * qd] = (midsb, 0)
                        midhist[2 * qd + 1] = (midsb, 1)
                        for old in (2 * qd - 8, 2 * qd - 7):
                            midhist.pop(old, None)

                    # ---- out quad: units v..v+3 (v%4==0) at round v+19 ----
                    v = r - 19
                    if 0 <= v < units and v % 4 == 0:
                        if v % group == 0:
                            outn_t = opool.tile([128, jg * 16], f32, tag="outn")
                        psout = ps_o.tile([128, 256], bf16, tag="o")
                        for up in range(4):
                            mt, mh = midhist[v + up + 14]
                            mc = 512 * mh
                            for b in range(4):
                                nc.tensor.transpose(
                                    psout[:, 64 * b + 16 * up : 64 * b + 16 * up + 16],
                                    mt[96:112, mc + 128 * b : mc + 128 * (b + 1)],
                                    i16b[96:112, :],
                                    tile_position=(96, 0),
                                )
                        # outn col = j*16+f, j = S*32 + b*8 + j2
                        s_v, j2 = v // SUPER, v % SUPER
                        dst = outn_t[:].rearrange(
                            "p (S b j2 f) -> p S b j2 f", b=4, j2=8, f=16
                        )[:, s_v % nsup, :, j2 : j2 + 4, :]
                        src = psout[:].rearrange("p (b u f) -> p b u f", b=4, u=4)
                        if (v // 4) % 3 != 2:
                            nc.scalar.activation(dst, src, COPY)
                        else:
                            nc.vector.tensor_copy(dst, src)
                        go = v // group
                        if go == ngroups - 1 and ngroups > 1 and group > SUPER:
                            if (v + 3) % SUPER == SUPER - 1:
                                sm = ((v + 3) % group) // SUPER
                                nc.sync.dma_start(
                                    out_view[go][:, 512 * sm : 512 * (sm + 1)],
                                    outn_t[:, 512 * sm : 512 * (sm + 1)],
                                )
                        elif (v + 3) % group == group - 1:
                            nc.sync.dma_start(out_view[go], outn_t[:])

    nc.compile()
    return nc


def _prep_weights(W1, b1, W2, b2, W3, b3, W4, b4):
    import ml_dtypes

    bf16 = ml_dtypes.bfloat16
    A1 = np.zeros((16, 128), np.float32)
    for n in range(8):
        A1[2 * n : 2 * n + 2, 16 * n : 16 * n + 16] = W1[n]
    a1lo = np.zeros((128, 128), np.float32)
    a1hi = np.zeros((128, 128), np.float32)
    for g in range(4):
        a1lo[32 * g : 32 * g + 16, :] = A1
        a1hi[32 * g + 16 : 32 * g + 32, :] = A1
    A2 = np.zeros((128, 64), np.float32)
    for n in range(4):
        A2[32 * n : 32 * n + 32, 16 * n : 16 * n + 16] = W2[n]
    A34 = np.zeros((96, 48), np.float32)
    for n in range(2):
        A34[32 * n : 32 * n + 32, 16 * n : 16 * n + 16] = W3[n]
    A34[64:96, 32:48] = W4
    b1v = np.ascontiguousarray(b1.reshape(128, 1), dtype=np.float32)
    b2v = np.concatenate(
        [b2.reshape(-1), b3.reshape(-1), b4.reshape(-1)]
    ).reshape(112, 1).astype(np.float32)
    return {
        "a1lo": a1lo.astype(bf16),
        "a1hi": a1hi.astype(bf16),
        "a2": A2.astype(bf16),
        "a34": A34.astype(bf16),
        "i16": np.eye(16, dtype=np.float32).astype(bf16),
        "i128": np.eye(128, dtype=np.float32).astype(bf16),
        "b1v": b1v,
        "b2v": b2v,
    }


def kernel(x, W1, b1, W2, b2, W3, b3, W4, b4, **_unused):
    from concourse.bass_utils import run_bass_kernel_spmd

    if "nc" not in _CACHE:
        _CACHE["nc"] = _build_nc()
    nc = _CACHE["nc"]

    x = np.ascontiguousarray(np.asarray(x, dtype=np.float32))
    wmap = _prep_weights(
        np.asarray(W1, np.float32),
        np.asarray(b1, np.float32),
        np.asarray(W2, np.float32),
        np.asarray(b2, np.float32),
        np.asarray(W3, np.float32),
        np.asarray(b3, np.float32),
        np.asarray(W4, np.float32),
        np.asarray(b4, np.float32),
    )
    in_maps = [
        {"x": x[i * BC : (i + 1) * BC], **wmap} for i in range(NCORES)
    ]
    _CACHE["in_maps"] = in_maps
    res = run_bass_kernel_spmd(nc, in_maps, core_ids=list(range(NCORES)))
    _CACHE["last"] = res
    out = np.concatenate([res.results[i]["out"] for i in range(NCORES)], axis=0)
    return out.astype(np.float32)

